# revision 4
# baseline (speedup 1.0000x reference)
"""Binary conv2d (XNOR-style) + per-channel scale for Trainium2 — v2.

y = conv2d(sign(x), sign(w), stride=1, pad=1) * scale[oc]

Data-parallel over batch across 8 NeuronCores (4 images each).  The 3x3
conv over 256 in-channels is accumulating fp8 DoubleRow matmuls (K=256)
into a PSUM tile per 8-output-row chunk, using shifted windows of a
zero-padded 57-column-stride image.  PSUM accumulates in fp32 and all
matmul inputs are exactly representable, so the result is bit-identical
to the fp32 reference.

v2 vs v1 (trace-driven, baseline 126986 ns):
  - the matmul stream is already at ~97% of the fp8-DR peak (195 ns
    issue-to-issue vs the 190 ns floor of 456 cycles @2.4 GHz), so all
    changes attack the startup (first real matmul was at 21.5 us) and
    tail;
  - image 0 is loaded and signed in four 14-row PIECES (both ic-slots
    per piece, one ACT sign each) so the first real matmul can start at
    ~12 us instead of waiting for full 28-row bands serialized behind
    weight DMAs;
  - the PE-clock warmup (1.2 -> 2.4 GHz needs ~3 us of continuous PE
    execution) now runs K=2 matmuls off a tiny [2, 584] scratch whose
    memset is ~0.6 us (v1's [128,2,592] scratch memset on gpsimd took
    1.1 us and pushed the first warmup to 7.9 us), and the warmup count
    is sized so warmups END right when piece-0's sign completes — v1's
    48 warmups overshot data-ready by ~9 us and then let the PE idle
    2.5 us (which also resets the PE clock to 1.2 GHz for ~3 us);
  - first compute groups are fine-grained (chunk 0 alone, then 2-chunk
    groups) and each is emitted right after the one sign it needs, so
    Tile's dependency tracking cannot over-wait on later signs;
  - the last image's final group is a single chunk so the post-stream
    drain+store tail is minimal.
"""

import numpy as np
import ml_dtypes

N_CORES = 8
IMGS = 4  # images per core
IC = 256
OC = 256
H = W = 56
# Padded row stride is 57, not 58: for a 3-wide kernel the left pad of
# row r+1 doubles as the right pad of row r, halving the dead columns.
WPAD = 57
XPAD_F = 3312  # 58 padded rows * 57 = 3306 -> pad to mult of 16
ROWS = 8  # output rows per PSUM tile
NFREE = ROWS * WPAD  # 456 <= 512 (PSUM bank limit)
NCHUNK = H // ROWS  # 7
PIECE = 14  # input rows per load/sign piece (4 pieces per image)
N_PIECES = H // PIECE
N_WARM = 18  # PE-clock warmup matmuls (~3us ramp + land at data-ready)

_cache = {}


def _install_drain_patch():
    """This walrus build rejects >1 sync-wait on ctrl-type instructions;
    Tile's kernel-tail drain carries one wait per pending proc.  Split it
    into one drain per proc (each with <=1 wait)."""
    import concourse.tile as _tile
    from concourse.vector_clock import ScopedClock, VectorClock

    if getattr(_tile.TileContext, "_drain_split_patch", False):
        return

    def _drain_and_barrier(self, tick_clock, wait_clock):
        nc = self.nc
        gclock = tick_clock.global_clock
        n = len(gclock)
        for p in range(n):
            t = gclock[p]
            if t <= 0:
                continue
            vec = [0] * n
            vec[p] = t
            d = nc.gpsimd.drain()
            wait_clock.add_sem_waits(d.ins, ScopedClock({None: VectorClock(vec)}))
        assert self.sems is not None
        popped = nc._tile_sem_poison_stack.pop()
        assert popped is self._sem_poison
        nc.clear_and_free_semaphores(list(self.sems.allocated().values()))

    _tile.TileContext._drain_and_barrier = _drain_and_barrier
    _tile.TileContext._drain_split_patch = True


def _split_excess_waits(nc, maxw=1):
    """Same walrus limitation: hoist excess sync-waits onto same-engine
    NoOps inserted just before the instruction (engine streams are
    in-order, so a preceding NoOp carrying the waits is equivalent)."""
    import concourse.mybir as mybir

    n_split = 0
    for f in nc.m.functions:
        for bb in f.blocks:
            out = []
            for ins in bb.instructions:
                si = ins.sync_info
                if si and si.on_wait and len(si.on_wait) > maxw:
                    waits = list(si.on_wait)
                    excess, keep = waits[:-maxw], waits[-maxw:]
                    for i in range(0, len(excess), maxw):
                        nop = mybir.InstNoOp(
                            name=f"{ins.name}_waitsplit{i}",
                            engine=ins.engine,
                            ins=[],
                            outs=[],
                            sync_info=mybir.SyncInfo(
                                on_wait=excess[i : i + maxw], on_update=[]
                            ),
                        )
                        out.append(nop)
                    si.on_wait = keep
                    n_split += 1
                out.append(ins)
            bb.instructions = out
    return n_split


def build_nc():
    import concourse.bass as bass
    import concourse.mybir as mybir
    from concourse.tile import TileContext

    _install_drain_patch()

    f32 = mybir.dt.float32
    fp8 = mybir.dt.float8e4
    DR = mybir.MatmulPerfMode.DoubleRow

    nc = bass.Bass()
    x = nc.declare_dram_parameter("x", [IMGS, IC, H, W], f32, isOutput=False)
    wb8 = nc.declare_dram_parameter("wb8", [128, 18, OC], fp8, isOutput=False)
    sc2 = nc.declare_dram_parameter("sc2", [128, 2], f32, isOutput=False)
    y = nc.declare_dram_parameter("y", [IMGS, OC, H, W], f32, isOutput=True)

    with TileContext(nc) as tc:
        with (
            tc.tile_pool(name="const", bufs=1) as cpool,
            tc.tile_pool(name="xin", bufs=4) as xin_pool,
            tc.tile_pool(name="outp", bufs=6) as out_pool,
            tc.tile_pool(name="psum", bufs=8, space="PSUM") as psum_pool,
        ):
            wb = cpool.tile([128, 18, OC], fp8)
            sc = cpool.tile([128, 2], f32)
            xp = cpool.tile([128, IMGS * 2, XPAD_F], fp8)

            # --- PE clock warmup scratch: K=2 matmuls only need a [2, X]
            # region; the memset is ~0.6us on DVE so the first warmup can
            # issue at ~6.7us (v1's 128-partition scratch cost 1.1us on
            # gpsimd and delayed the warmups to 7.9us).
            wsc = cpool.tile([2, 584], fp8)
            nc.vector.memset(wsc[:], 0.0)

            # --- startup DMAs for image 0, piecewise.  Ring order == issue
            # order == delivery order; sequence strictly by NEED:
            #   sync ring:   p0s0, p1s0, p1s1, p2s0, p3s0
            #   scalar ring: sc, p0s1, wb[0:6], wb[6:18], p2s1, p3s1
            # (p1s1 rides sync so piece 1 completes before the second
            # compute group needs it; weights interleave on scalar between
            # piece-0 and piece-1 because the first LDWEIGHTS is only
            # needed once piece 0 is signed.)
            def piece_tile(n, p):
                return xin_pool.tile(
                    [128, 2, PIECE, W], f32, name=f"xin{n}_{p}", tag="xin"
                )

            p_tiles = [piece_tile(0, p) for p in range(N_PIECES)]
            r = [(p * PIECE, (p + 1) * PIECE) for p in range(N_PIECES)]
            nc.sync.dma_start(out=p_tiles[0][:, 0], in_=x[0, 0:128, r[0][0]:r[0][1], :])
            nc.scalar.dma_start(out=sc[:], in_=sc2[:, :])
            nc.scalar.dma_start(
                out=p_tiles[0][:, 1], in_=x[0, 128:256, r[0][0]:r[0][1], :]
            )
            nc.sync.dma_start(out=p_tiles[1][:, 0], in_=x[0, 0:128, r[1][0]:r[1][1], :])
            nc.scalar.dma_start(out=wb[:, 0:6, :], in_=wb8[:, 0:6, :])
            nc.sync.dma_start(
                out=p_tiles[1][:, 1], in_=x[0, 128:256, r[1][0]:r[1][1], :]
            )
            nc.scalar.dma_start(out=wb[:, 6:18, :], in_=wb8[:, 6:18, :])
            nc.sync.dma_start(out=p_tiles[2][:, 0], in_=x[0, 0:128, r[2][0]:r[2][1], :])
            nc.scalar.dma_start(
                out=p_tiles[2][:, 1], in_=x[0, 128:256, r[2][0]:r[2][1], :]
            )
            nc.sync.dma_start(out=p_tiles[3][:, 0], in_=x[0, 0:128, r[3][0]:r[3][1], :])
            nc.scalar.dma_start(
                out=p_tiles[3][:, 1], in_=x[0, 128:256, r[3][0]:r[3][1], :]
            )

            # --- PE clock warmup.  The tensor engine ramps 1.2 -> 2.4 GHz
            # only after ~3us of CONTINUOUS execution; without this the
            # first ~13 real matmuls run at half clock.  Keep the PE busy
            # on throwaway K=2 matmuls (456-wide like the real stream) for
            # the startup-delivery window; the count is tuned so the last
            # warmup retires right as piece 0's sign completes — overshoot
            # delays the real stream, undershoot lets the clock reset.
            for k in range(N_WARM):
                ps = psum_pool.tile([128, NFREE], f32, name=f"warm{k}", tag="ps")
                nc.tensor.matmul(
                    ps[:], wsc[:, 456:584], wsc[:, 0:NFREE],
                    start=True, stop=True,
                )

            def pad_ring(j):
                # zero only the padding ring (interior is overwritten by
                # the sign): top pad row; each data row's col 0 (also the
                # previous row's right pad); bottom pad row + tail.
                eng = nc.vector if j % 2 == 0 else nc.gpsimd
                xpj = xp[:, j, :]
                eng.memset(xpj[:, 0:WPAD], 0.0)
                lefts = xpj[:, WPAD : WPAD + H * WPAD].rearrange(
                    "p (r c) -> p r c", c=WPAD
                )[:, :, 0:1]
                eng.memset(lefts, 0.0)
                eng.memset(xpj[:, (H + 1) * WPAD : XPAD_F], 0.0)

            pad_ring(0)
            pad_ring(1)

            def sign_piece(n, p, tile):
                # binarize both ic-slots of one 14-row piece to +-1 via the
                # ACT sign activation (signs own ACT; drains own DVE).
                r0, r1 = p * PIECE, (p + 1) * PIECE
                base = (r0 + 1) * WPAD + 1
                dst = (
                    xp[:, 2 * n : 2 * n + 2, base : base + (r1 - r0) * WPAD]
                    .rearrange("p j (h w) -> p j h w", w=WPAD)[:, :, :, 0:W]
                )
                nc.scalar.sign(dst, tile[:])

            def compute_image(n, subs):
                # tap-outer (weight-stationary) so consecutive matmuls hit
                # different PSUM banks.  LDWEIGHTS overlaps MATMUL via the
                # PE dual weight buffer.
                for c0, c1 in subs:
                    for ocb in range(2):
                        psums = [
                            psum_pool.tile(
                                [128, NFREE], f32, name=f"ps{n}{ocb}{c}", tag="ps"
                            )
                            for c in range(c0, c1)
                        ]
                        for t in range(9):
                            kh, kw = divmod(t, 3)
                            lhsT = wb[:, 2 * t : 2 * t + 2, ocb * 128 : (ocb + 1) * 128]
                            rhs_slot = xp[:, 2 * n : 2 * n + 2, :]
                            for c in range(c0, c1):
                                off = c * ROWS * WPAD + kh * WPAD + kw
                                nc.tensor.matmul(
                                    psums[c - c0][:],
                                    lhsT,
                                    rhs_slot[:, :, off : off + NFREE],
                                    start=(t == 0),
                                    stop=(t == 8),
                                    perf_mode=DR,
                                )
                        for c in range(c0, c1):
                            out_c = out_pool.tile([128, ROWS, W], f32)
                            src = psums[c - c0].rearrange("p (h w) -> p h w", w=WPAD)[
                                :, :, 0:W
                            ]
                            # all drains on DVE (signs own ACT; Pool cannot
                            # read PSUM); the fp32 scale is applied here
                            nc.vector.tensor_scalar_mul(
                                out_c[:], src, sc[:, ocb : ocb + 1]
                            )
                            nc.sync.dma_start(
                                out=y[n, ocb * 128 : (ocb + 1) * 128, c * ROWS : (c + 1) * ROWS, :],
                                in_=out_c[:],
                            )

            # image 0: each fine-grained compute group is emitted right
            # after the ONE sign it needs, so dependency tracking cannot
            # over-wait on later signs.  chunk c needs input rows
            # [8c-1, 8c+9) -> group (0,1) needs piece 0; (1,3) pieces 0-1;
            # (3,5) pieces 1-2; (5,7) pieces 2-3.
            sign_piece(0, 0, p_tiles[0])
            compute_image(0, subs=((0, 1),))
            sign_piece(0, 1, p_tiles[1])
            compute_image(0, subs=((1, 3),))
            sign_piece(0, 2, p_tiles[2])
            compute_image(0, subs=((3, 5),))
            sign_piece(0, 3, p_tiles[3])

            def load_image_pads_signs(n):
                # both slots of each piece ride the Scalar-engine HWDGE
                # ring; the xin pool rotation (bufs=4) paces piece k of
                # image n behind the sign that releases piece k of image
                # n-1, so these transfers never steal DMA bandwidth from
                # the startup-critical loads.
                pad_ring(2 * n)
                pad_ring(2 * n + 1)
                for p in range(N_PIECES):
                    t = piece_tile(n, p)
                    r0, r1 = p * PIECE, (p + 1) * PIECE
                    nc.scalar.dma_start(out=t[:, 0], in_=x[n, 0:128, r0:r1, :])
                    nc.scalar.dma_start(out=t[:, 1], in_=x[n, 128:256, r0:r1, :])
                    sign_piece(n, p, t)

            # interleave: image n+1's loads/signs are emitted between image
            # n's compute subgroups; the pool rotation paces the DMAs.
            compute_image(0, subs=((5, NCHUNK),))
            load_image_pads_signs(1)
            compute_image(1, subs=((0, 4),))
            load_image_pads_signs(2)
            compute_image(1, subs=((4, NCHUNK),))
            compute_image(2, subs=((0, 4),))
            load_image_pads_signs(3)
            compute_image(2, subs=((4, NCHUNK),))
            # final group is a single chunk so the drain+store tail after
            # the last matmul is as short as possible
            compute_image(3, subs=((0, 3), (3, 5), (5, 6), (6, NCHUNK)))

    _split_excess_waits(nc)
    return nc


def _get_nc():
    if "nc" not in _cache:
        _cache["nc"] = build_nc()
    return _cache["nc"]


def _prep_weights(weight, scale):
    # host-side: binarize weights, lay out [p, (kh kw icb), oc] fp8; the
    # per-channel scale is rearranged to [p, ocb].
    w = np.asarray(weight, dtype=np.float32)  # [oc, ic, kh, kw]
    wb = np.sign(w).transpose(2, 3, 1, 0)  # [kh, kw, ic, oc]
    wb = wb.reshape(3, 3, 2, 128, OC).transpose(3, 0, 1, 2, 4).reshape(128, 18, OC)
    wb8 = np.ascontiguousarray(wb).astype(ml_dtypes.float8_e4m3)
    sc2 = np.ascontiguousarray(np.asarray(scale, dtype=np.float32).reshape(2, 128).T)
    return wb8, sc2


def run(inputs, trace=False, trace_cores=None):
    from concourse.bass_utils import run_bass_kernel_spmd

    x = np.asarray(inputs["x"])
    wb8, sc2 = _prep_weights(inputs["weight"], inputs["scale"])

    in_maps = [
        {"x": x[i * IMGS : (i + 1) * IMGS], "wb8": wb8, "sc2": sc2}
        for i in range(N_CORES)
    ]
    res = run_bass_kernel_spmd(
        _get_nc(),
        in_maps,
        core_ids=list(range(N_CORES)),
        trace=trace,
        trace_cores=trace_cores,
    )
    out = np.concatenate([res.results[i]["y"] for i in range(N_CORES)], axis=0)
    return out, res


def kernel(**inputs):
    # One retry: a previously crashed process can leave a core wedged
    # (NRT_EXEC_UNIT_UNRECOVERABLE); the runtime recovers on the next
    # attempt.
    try:
        out, _ = run(inputs, trace=False)
    except Exception:
        out, _ = run(inputs, trace=False)
    return out
